# revision 24
# baseline (speedup 1.0000x reference)
"""Causal multi-head self-attention (RoPE on input) for Trainium2, 8 NeuronCores.

Sharding: core c handles batch b = c//2 and head-group g = c%2 (8 of 16 heads).
Wq/Wk/Wv are split column-wise per head-group, Wo row-wise; each core produces a
partial (T, E) output (bf16) and the host sums the two head-group partials per
batch and adds the bias.

Device layout notes:
- Activations are kept transposed (feature dim on partitions) so every matmul
  contraction runs over the partition dim with no on-device transposes.
- The input is passed de-interleaved (even RoPE pair lanes then odd lanes) so
  the RoPE pair swap is partition-aligned; the Wq/Wk/Wv rows carry the same
  permutation.
- Softmax is computed without max-subtraction (scores are O(+-10) for this
  distribution, exp is safe in fp32); the normalizer comes from a ones column
  appended to V.
- All input DMAs are chunked and priority-ordered (wv chunks, then
  xt/sin/cos packs per rope pair, masks, per-pair wk/wq slices, wo) so the
  V projection and rope stream behind the DMA front instead of waiting for
  whole-tensor loads. The V projection runs contraction-outer over two
  groups of 8 PSUM banks so each chunk is consumed as it lands.
- Causal diagonal blocks only compute the live suffix: both the scores and
  the PV matmuls slice their free dim to skip fully-masked columns. Within
  each 512-query cell the diagonal blocks run FIRST (their suffix exps are
  small, so the ACT pipeline ramps quickly); PV accumulation runs
  off-diagonal-first so the full-width block carries start=True.
- Softmax normalizers are packed into rows via small DMAs and inverted with
  batched DVE reciprocals (the reciprocal op is ~6.5 cycles/elem, so it
  must never sit in front of attention-critical DVE work): pairs 0-2 defer
  their reciprocal+scale until just before the output projection (lowest
  priority, soaked up by idle DVE), while pair 3 normalizes cells eagerly
  ({0,1} at i==1, {2} at i==2) so output-projection tiles unlock as PE
  filler during the final pair's exp-bound stretch.
- The output projection shares the qk PSUM pool and overlaps the last
  pair's attention via the dataflow scheduler; output is written bf16 and
  summed on the host in fp32.
"""

import numpy as np
import ml_dtypes

import concourse.bacc as bacc
import concourse.tile as tile
import concourse.mybir as mybir
from concourse import bass_utils
from concourse.bass_interp import get_hw_module

bf16 = ml_dtypes.bfloat16
BF = mybir.dt.bfloat16
F32 = mybir.dt.float32
EXP = mybir.ActivationFunctionType.Exp

B, T, E = 4, 2048, 1024
H, HD = 16, 64
G = 2  # head groups (tensor-parallel dimension)
HL = H // G  # heads per core
DL = HL * HD  # 512 local feature dim
P = 128
NT = T // P  # 16 tk tiles
NQ = T // 512  # 4 tq tiles
EC = E // P  # 8 contraction chunks over E
DC = DL // P  # 4 chunks over local head dims

# chunk order matching rope pair arrival (u, u+4)
JSEQ = [0, 4, 1, 5, 2, 6, 3, 7]

_CACHE = {}
LAST_RESULT = None


def _build():
    nc = bacc.Bacc("TRN2", target_bir_lowering=False, debug=False, num_devices=8)
    xt_d = nc.dram_tensor("xt", (EC, P, T), BF, kind="ExternalInput").ap()
    sinh_d = nc.dram_tensor("sinh", (4, P, T), BF, kind="ExternalInput").ap()
    cosh_d = nc.dram_tensor("cosh", (4, P, T), BF, kind="ExternalInput").ap()
    wq_d = nc.dram_tensor("wq", (EC, P, DL), BF, kind="ExternalInput").ap()
    wk_d = nc.dram_tensor("wk", (EC, P, DL), BF, kind="ExternalInput").ap()
    wv_d = nc.dram_tensor("wv", (EC, P, DL), BF, kind="ExternalInput").ap()
    wo_d = nc.dram_tensor("wo", (DC, P, E), BF, kind="ExternalInput").ap()
    wqv = wq_d.rearrange("o p n -> p o n")
    wkv = wk_d.rearrange("o p n -> p o n")
    masks_d = nc.dram_tensor("masks", (P, 512), BF, kind="ExternalInput").ap()
    out_d = nc.dram_tensor("out", (T, E), BF, kind="ExternalOutput").ap()


    with tile.TileContext(nc) as tc:
        with tc.tile_pool(name="persist", bufs=1) as persist:
            rx = persist.tile([P, EC, T], BF)
            qT = persist.tile([P, DC, T], BF)
            kT = persist.tile([P, DC, T], BF)
            v = persist.tile([P, NT, HL, HD + 1], BF)
            oc = persist.tile([P, DC, T], BF)
            wq = persist.tile([P, EC, DL], BF)
            wk = persist.tile([P, EC, DL], BF)
            wv = persist.tile([P, EC, DL], BF)
            masks = persist.tile([P, 512], BF)
            nc.vector.memset(v[:, :, :, HD : HD + 1], 1.0)

            # ---- Phase A: RoPE + V projection, streamed behind the DMAs ----
            with (
                tc.tile_pool(name="xtp", bufs=1) as xtp,
                tc.tile_pool(name="tabs", bufs=1) as tabs,
                tc.tile_pool(name="tmps", bufs=1) as tmps,
                tc.tile_pool(name="mmf", bufs=1, space="PSUM") as mmf,
                tc.tile_pool(name="mm1", bufs=1, space="PSUM") as mm1,
            ):
                filler = mmf.tile([P, 1024], F32)
                xt = xtp.tile([P, EC, T], BF)
                sin_t = tabs.tile([P, 4, T], BF)
                cos_t = tabs.tile([P, 4, T], BF)

                # DMA priority order: wv chunks, (xt pair + tables) packs,
                # masks, per-pair wk/wq, wo (emitted in phase B/C block).
                for u in range(4):
                    nc.sync.dma_start(wv[:, u, :], wv_d[u])
                    nc.sync.dma_start(xt[:, u, :], xt_d[u])
                    nc.sync.dma_start(wv[:, u + 4, :], wv_d[u + 4])
                    nc.sync.dma_start(xt[:, u + 4, :], xt_d[u + 4])
                    nc.sync.dma_start(sin_t[:, u, :], sinh_d[u])
                    nc.sync.dma_start(cos_t[:, u, :], cosh_d[u])
                nc.sync.dma_start(masks, masks_d)
                for hp in range(HL // 2):
                    nc.sync.dma_start(
                        wk[:, :, P * hp : P * (hp + 1)],
                        wkv[:, :, P * hp : P * (hp + 1)],
                    )
                    nc.sync.dma_start(
                        wq[:, :, P * hp : P * (hp + 1)],
                        wqv[:, :, P * hp : P * (hp + 1)],
                    )

                # rope on DVE, one (u, u+4) chunk pair at a time
                for u in range(4):
                    xe = xt[:, u, :]
                    xo = xt[:, u + 4, :]
                    st = sin_t[:, u, :]
                    ct = cos_t[:, u, :]
                    t1 = tmps.tile([P, T], BF, tag="t1")
                    nc.vector.tensor_mul(t1, xe, ct)
                    t2 = tmps.tile([P, T], BF, tag="t2")
                    nc.vector.tensor_mul(t2, xo, st)
                    nc.vector.tensor_sub(rx[:, u, :], t1, t2)
                    t3 = tmps.tile([P, T], BF, tag="t3")
                    nc.vector.tensor_mul(t3, xo, ct)
                    t4 = tmps.tile([P, T], BF, tag="t4")
                    nc.vector.tensor_mul(t4, xe, st)
                    nc.vector.tensor_add(rx[:, u + 4, :], t3, t4)

                # V projection: contraction-outer over 2 groups of 8 tk
                # tiles so each xt chunk is consumed as it arrives
                for grp, tks in enumerate((range(0, 6), range(6, 12), range(12, 16))):
                    vps = {}
                    for tk in tks:
                        vps[tk] = mm1.tile(
                            [P, DL], F32, tag=f"vp{tk % 6}", name=f"vp{tk % 6}"
                        )
                    for idx, j in enumerate(JSEQ):
                        for tk in tks:
                            nc.tensor.matmul(
                                vps[tk],
                                lhsT=xt[:, j, P * tk : P * (tk + 1)],
                                rhs=wv[:, j, :],
                                start=(idx == 0),
                                stop=(idx == 7),
                            )
                    for tk in tks:
                        if tk % 2 == 0:
                            nc.scalar.copy(
                                v[:, tk, :, 0:HD],
                                vps[tk].rearrange("p (h d) -> p h d", h=HL),
                            )
                        else:
                            nc.vector.tensor_copy(
                                v[:, tk, :, 0:HD],
                                vps[tk].rearrange("p (h d) -> p h d", h=HL),
                            )

            # ---- Phase B+C: Q/K projection, attention, output projection ----
            with (
                tc.tile_pool(name="att", bufs=10) as apool,
                tc.tile_pool(name="norm", bufs=2) as npool,
                tc.tile_pool(name="rbp", bufs=1) as rbpool,
                tc.tile_pool(name="wop", bufs=1) as wop,
                tc.tile_pool(name="oout", bufs=3) as oop,
                tc.tile_pool(name="qkps", bufs=2, space="PSUM") as qkps,
                tc.tile_pool(name="ops", bufs=2, space="PSUM") as opool,
                tc.tile_pool(name="sps", bufs=2, space="PSUM") as spool,
                tc.tile_pool(name="dramn", bufs=1, space="DRAM") as dpool,
            ):
                rpk_d = dpool.tile([HL // 2, 8, 512], BF)
                wo = wop.tile([P, DC, E], BF)
                nc.sync.dma_start(wo, wo_d.rearrange("o p n -> p o n"))

                deferred_norm = []
                for hp in range(HL // 2):
                    h0, h1 = 2 * hp, 2 * hp + 1
                    rbtag = f"rbd{hp % 2}" if hp < 3 else "rb3"
                    rb = rbpool.tile([P, T], BF, tag=rbtag, name=f"rb{hp}")
                    if hp < 3:
                        packed_a = npool.tile(
                            [8, 512], BF, tag=f"packed_a{hp}", name=f"packeda{hp}"
                        )
                    else:
                        packed_a = npool.tile(
                            [4, 512], BF, tag="packed_g0", name=f"packedg0{hp}"
                        )
                        packed_g1 = npool.tile(
                            [2, 512], BF, tag="packed_g1", name=f"packedg1{hp}"
                        )
                    packed_b = npool.tile(
                        [2, 512], BF, tag="packed_b", name=f"packedb{hp}"
                    )

                    # Q/K projection for this head pair, ti-outer so the
                    # first attention cell unblocks as early as possible
                    for ti in range(NQ):
                        for w_sb, dst in ((wk, kT), (wq, qT)):
                            pp = qkps.tile([P, 512], F32, tag="qk")
                            for idx, j in enumerate(JSEQ):
                                nc.tensor.matmul(
                                    pp,
                                    lhsT=w_sb[:, j, P * hp : P * (hp + 1)],
                                    rhs=rx[:, j, 512 * ti : 512 * (ti + 1)],
                                    start=(idx == 0),
                                    stop=(idx == 7),
                                )
                            if hp == 0:
                                nc.scalar.copy(
                                    dst[:, hp, 512 * ti : 512 * (ti + 1)], pp
                                )
                            else:
                                nc.vector.tensor_copy(
                                    dst[:, hp, 512 * ti : 512 * (ti + 1)], pp
                                )

                    for i in range(NQ):
                        nj = 4 * i + 4
                        tq = slice(512 * i, 512 * (i + 1))
                        op0 = opool.tile([HD + 1, 512], F32, tag="o")
                        op1 = opool.tile([HD + 1, 512], F32, tag="o")
                        ats = {}

                        def emit_pv(jp, first, last):
                            at_jp = ats[jp]
                            r = jp - 4 * i
                            lo = P * r if r >= 1 else 0
                            nc.tensor.matmul(
                                op0[:, lo:],
                                lhsT=v[:, jp, h0, :],
                                rhs=at_jp[:, 0, lo:],
                                start=first,
                                stop=last,
                            )
                            nc.tensor.matmul(
                                op1[:, lo:],
                                lhsT=v[:, jp, h1, :],
                                rhs=at_jp[:, 1, lo:],
                                start=first,
                                stop=last,
                            )

                        # diagonal blocks first: their suffix exps are small,
                        # so the ACT pipeline ramps quickly at cell start.
                        # PV accumulation runs off-diagonal first (full-width
                        # start) and diagonals last.
                        sorder = list(range(4 * i, nj)) + list(range(0, 4 * i))
                        pvorder = list(range(0, 4 * i)) + list(range(4 * i, nj))
                        npv = 0
                        for idx, j in enumerate(sorder):
                            r = j - 4 * i
                            lo = P * r if r >= 1 else 0
                            sp = spool.tile([P, 2, 512], F32, tag="s")
                            nc.tensor.matmul(
                                sp[:, 0, lo:],
                                lhsT=kT[0:HD, hp, P * j : P * (j + 1)],
                                rhs=qT[0:HD, hp, 512 * i + lo : 512 * (i + 1)],
                                start=True,
                                stop=True,
                            )
                            nc.tensor.matmul(
                                sp[:, 1, lo:],
                                lhsT=kT[HD:P, hp, P * j : P * (j + 1)],
                                rhs=qT[HD:P, hp, 512 * i + lo : 512 * (i + 1)],
                                start=True,
                                stop=True,
                            )
                            at = apool.tile([P, 2, 512], BF, tag="a")
                            if r < 0:
                                nc.scalar.activation(at, sp, EXP, scale=0.125)
                            else:
                                w = 512 - lo
                                nc.scalar.activation(
                                    at[:, :, lo:], sp[:, :, lo:], EXP, scale=0.125
                                )
                                # big cells: mask on the idle Pool engine --
                                # diagonal at tiles aren't read until the
                                # cell-end PV tail, so Pool latency is hidden
                                # and DVE contention drops
                                eng = nc.gpsimd if i >= 2 else nc.vector
                                eng.tensor_mul(
                                    at[:, :, lo:],
                                    at[:, :, lo:],
                                    masks[:, None, 0:w].to_broadcast((P, 2, w)),
                                )
                            ats[j] = at
                            # trail PV ~3 ready blocks behind; only emit PVs
                            # whose at exists (off-diags become available
                            # from idx 4 onward)
                            while npv < len(pvorder) and pvorder[npv] in ats:
                                jp = pvorder[npv]
                                if i > 0 and jp >= 4 * i and idx < nj - 1:
                                    break  # diags last; wait until loop end
                                if idx - 3 < npv:
                                    break
                                emit_pv(jp, npv == 0, npv == nj - 1)
                                npv += 1
                        while npv < len(pvorder):
                            emit_pv(pvorder[npv], npv == 0, npv == nj - 1)
                            npv += 1

                        # normalizer rows: copy out, pack via DMA; batch
                        # the expensive DVE reciprocal (cells 0-2 at i==2,
                        # cell 3 alone) and scale oc in two chunks so the
                        # output projection overlaps the pair tail
                        s0 = npool.tile([1, 512], BF, tag="s0")
                        nc.vector.tensor_copy(s0, op0[HD : HD + 1, :])
                        s1 = npool.tile([1, 512], BF, tag="s1")
                        nc.vector.tensor_copy(s1, op1[HD : HD + 1, :])
                        if hp < 3:
                            nc.sync.dma_start(packed_a[i : i + 1, :], s0)
                            nc.sync.dma_start(packed_a[4 + i : 5 + i, :], s1)
                        elif i < 2:
                            nc.sync.dma_start(packed_a[2 * i : 2 * i + 1, :], s0)
                            nc.sync.dma_start(
                                packed_a[2 * i + 1 : 2 * i + 2, :], s1
                            )
                        elif i == 2:
                            nc.sync.dma_start(packed_g1[0:1, :], s0)
                            nc.sync.dma_start(packed_g1[1:2, :], s1)
                        else:
                            nc.sync.dma_start(packed_b[0:1, :], s0)
                            nc.sync.dma_start(packed_b[1:2, :], s1)
                        nc.vector.tensor_copy(oc[0:HD, hp, tq], op0[0:HD, :])
                        nc.vector.tensor_copy(oc[HD:P, hp, tq], op1[0:HD, :])

                        if hp == 3 and i in (1, 2):
                            # last pair: normalize finished cells eagerly so
                            # output-projection tiles unlock as PE filler
                            lo8, hi8 = (0, 4) if i == 1 else (4, 6)
                            gsrc = packed_a if i == 1 else packed_g1
                            rpk_a = npool.tile(
                                [hi8 - lo8, 512],
                                BF,
                                tag=f"rpk_a{i}",
                                name=f"rpka{hp}_{i}",
                            )
                            with nc.allow_low_precision(
                                reason="bf16 softmax normalizer"
                            ):
                                nc.vector.reciprocal(rpk_a, gsrc)
                            nc.sync.dma_start(rpk_d[hp, lo8:hi8], rpk_a)
                            for c in range(lo8 // 2, hi8 // 2):
                                tqi = slice(512 * c, 512 * (c + 1))
                                nc.sync.dma_start(
                                    rb[0:HD, tqi],
                                    rpk_d[hp, 2 * c : 2 * c + 1, :].to_broadcast(
                                        (HD, 512)
                                    ),
                                )
                                nc.sync.dma_start(
                                    rb[HD:P, tqi],
                                    rpk_d[
                                        hp, 2 * c + 1 : 2 * c + 2, :
                                    ].to_broadcast((HD, 512)),
                                )
                            nc.vector.tensor_mul(
                                oc[:, hp, 512 * (lo8 // 2) : 512 * (hi8 // 2)],
                                oc[:, hp, 512 * (lo8 // 2) : 512 * (hi8 // 2)],
                                rb[:, 512 * (lo8 // 2) : 512 * (hi8 // 2)],
                            )

                    if hp < 3:
                        # normalization deferred: emitted just before the
                        # output projection so its DVE cost never outranks
                        # the attention-critical mask multiplies
                        deferred_norm.append((hp, packed_a, rb))
                    else:
                        rpk_b = npool.tile(
                            [2, 512], BF, tag="rpk_b", name=f"rpkb{hp}"
                        )
                        with nc.allow_low_precision(reason="bf16 softmax normalizer"):
                            nc.vector.reciprocal(rpk_b, packed_b)
                        nc.sync.dma_start(rpk_d[hp, 6:8], rpk_b)
                        tq3 = slice(512 * 3, 512 * 4)
                        nc.sync.dma_start(
                            rb[0:HD, tq3],
                            rpk_d[hp, 6:7, :].to_broadcast((HD, 512)),
                        )
                        nc.sync.dma_start(
                            rb[HD:P, tq3],
                            rpk_d[hp, 7:8, :].to_broadcast((HD, 512)),
                        )
                        nc.vector.tensor_mul(
                            oc[:, hp, tq3], oc[:, hp, tq3], rb[:, tq3]
                        )

                for dhp, dpacked, drb in deferred_norm:
                    rpk_a = npool.tile(
                        [8, 512], BF, tag=f"rpk_a8_{dhp}", name=f"rpka8{dhp}"
                    )
                    with nc.allow_low_precision(reason="bf16 softmax normalizer"):
                        nc.vector.reciprocal(rpk_a, dpacked)
                    nc.sync.dma_start(rpk_d[dhp, 0:8], rpk_a)
                    for ii in range(4):
                        tqi = slice(512 * ii, 512 * (ii + 1))
                        nc.sync.dma_start(
                            drb[0:HD, tqi],
                            rpk_d[dhp, ii : ii + 1, :].to_broadcast((HD, 512)),
                        )
                        nc.sync.dma_start(
                            drb[HD:P, tqi],
                            rpk_d[dhp, 4 + ii : 5 + ii, :].to_broadcast(
                                (HD, 512)
                            ),
                        )
                    nc.vector.tensor_mul(oc[:, dhp, :], oc[:, dhp, :], drb)

                # ---- output projection (shares the qk PSUM pool; the
                # dataflow scheduler overlaps it with the last pair) ----
                for tt in range(NT):
                    ot = oop.tile([P, E], BF, tag="ot")
                    for et in range(2):
                        pp = qkps.tile([P, 512], F32, tag="qk")
                        for kk in range(DC):
                            nc.tensor.matmul(
                                pp,
                                lhsT=oc[:, kk, P * tt : P * (tt + 1)],
                                rhs=wo[:, kk, 512 * et : 512 * (et + 1)],
                                start=(kk == 0),
                                stop=(kk == DC - 1),
                            )
                        if et == 0:
                            nc.scalar.copy(ot[:, 512 * et : 512 * (et + 1)], pp)
                        else:
                            nc.vector.tensor_copy(
                                ot[:, 512 * et : 512 * (et + 1)], pp
                            )
                        nc.sync.dma_start(
                            out_d[
                                P * tt : P * (tt + 1), 512 * et : 512 * (et + 1)
                            ],
                            ot[:, 512 * et : 512 * (et + 1)],
                        )

    nc.compile()
    nc.m = get_hw_module(nc.m)
    return nc


def _prep_inputs(input, Wq, Wk, Wv, Wo):
    """Host-side shard prep: transpose/de-interleave/cast. Returns 8 in_maps."""
    perm = np.concatenate([np.arange(0, E, 2), np.arange(1, E, 2)])

    u = np.arange(E // 2, dtype=np.float64)
    thetas = 10000.0 ** (-2.0 * u / E)
    ang = np.arange(T, dtype=np.float64)[:, None] * thetas[None, :]
    sinh = np.sin(ang).T.reshape(4, P, T).astype(bf16)
    cosh = np.cos(ang).T.reshape(4, P, T).astype(bf16)

    masks = np.zeros((P, 512), np.float32)
    f = np.arange(512)
    for p in range(P):
        masks[p] = (f >= p).astype(np.float32)
    masks = masks.astype(bf16)

    xt = [
        np.ascontiguousarray(input[b].T[perm]).reshape(EC, P, T).astype(bf16)
        for b in range(B)
    ]
    WqT, WkT, WvT = Wq.T[perm], Wk.T[perm], Wv.T[perm]
    wq_g = [
        np.ascontiguousarray(WqT[:, DL * g : DL * (g + 1)])
        .reshape(EC, P, DL)
        .astype(bf16)
        for g in range(G)
    ]
    wk_g = [
        np.ascontiguousarray(WkT[:, DL * g : DL * (g + 1)])
        .reshape(EC, P, DL)
        .astype(bf16)
        for g in range(G)
    ]
    wv_g = [
        np.ascontiguousarray(WvT[:, DL * g : DL * (g + 1)])
        .reshape(EC, P, DL)
        .astype(bf16)
        for g in range(G)
    ]
    wo_g = [
        np.ascontiguousarray(Wo.T[DL * g : DL * (g + 1)])
        .reshape(DC, P, E)
        .astype(bf16)
        for g in range(G)
    ]

    in_maps = []
    for c in range(8):
        b, g = c // 2, c % 2
        in_maps.append(
            {
                "xt": xt[b],
                "sinh": sinh,
                "cosh": cosh,
                "wq": wq_g[g],
                "wk": wk_g[g],
                "wv": wv_g[g],
                "wo": wo_g[g],
                "masks": masks,
            }
        )
    return in_maps


def kernel(input, Wq, Wk, Wv, Wo, bo):
    global LAST_RESULT
    input = np.asarray(input, np.float32)
    Wq, Wk, Wv, Wo = (np.asarray(w, np.float32) for w in (Wq, Wk, Wv, Wo))
    bo = np.asarray(bo, np.float32)

    if "nc" not in _CACHE:
        _CACHE["nc"] = _build()
    nc = _CACHE["nc"]

    in_maps = _prep_inputs(input, Wq, Wk, Wv, Wo)
    res = bass_utils.run_bass_kernel_spmd(nc, in_maps, core_ids=list(range(8)))
    LAST_RESULT = res

    out = np.empty((B, T, E), np.float32)
    for b in range(B):
        out[b] = (
            res.results[2 * b]["out"].astype(np.float32)
            + res.results[2 * b + 1]["out"].astype(np.float32)
            + bo
        )
    return out


# revision 25
# speedup vs baseline: 1.1920x; 1.1920x over previous
"""Causal multi-head self-attention (RoPE on input) for Trainium2, 8 NeuronCores.

Sharding: core c handles batch b = c//2 and head-group g = c%2 (8 of 16 heads).
Wq/Wk/Wv are split column-wise per head-group, Wo row-wise; each core produces a
partial (T, E) output (bf16) and the host sums the two head-group partials per
batch and adds the bias.

Device layout notes:
- Activations are kept transposed (feature dim on partitions) so every matmul
  contraction runs over the partition dim with no on-device transposes.
- The input is passed de-interleaved (even RoPE pair lanes then odd lanes) so
  the RoPE pair swap is partition-aligned; the Wq/Wk/Wv rows carry the same
  permutation.
- Softmax is computed without max-subtraction (scores are O(+-10) for this
  distribution, exp is safe in fp32); the normalizer comes from a ones column
  appended to V.
- All input DMAs are chunked and priority-ordered (wv chunks, then
  xt/sin/cos packs per rope pair, masks, per-pair wk/wq slices, wo) so the
  V projection and rope stream behind the DMA front instead of waiting for
  whole-tensor loads. The V projection runs contraction-outer over two
  groups of 8 PSUM banks so each chunk is consumed as it lands.
- Causal diagonal blocks only compute the live suffix: both the scores and
  the PV matmuls slice their free dim to skip fully-masked columns. Within
  each 512-query cell the diagonal blocks run FIRST (their suffix exps are
  small, so the ACT pipeline ramps quickly); PV accumulation runs
  off-diagonal-first so the full-width block carries start=True.
- Softmax normalizers are packed into rows via small DMAs and inverted with
  batched DVE reciprocals (the reciprocal op is ~6.5 cycles/elem, so it
  must never sit in front of attention-critical DVE work): pairs 0-2 defer
  their reciprocal+scale until just before the output projection (lowest
  priority, soaked up by idle DVE), while pair 3 normalizes cells eagerly
  ({0,1} at i==1, {2} at i==2) so output-projection tiles unlock as PE
  filler during the final pair's exp-bound stretch.
- The output projection shares the qk PSUM pool and overlaps the last
  pair's attention via the dataflow scheduler; output is written bf16 and
  summed on the host in fp32.
"""

import numpy as np
import ml_dtypes

import concourse.bacc as bacc
import concourse.tile as tile
import concourse.mybir as mybir
from concourse import bass_utils
from concourse.bass_interp import get_hw_module

bf16 = ml_dtypes.bfloat16
BF = mybir.dt.bfloat16
F32 = mybir.dt.float32
EXP = mybir.ActivationFunctionType.Exp

B, T, E = 4, 2048, 1024
H, HD = 16, 64
G = 2  # head groups (tensor-parallel dimension)
HL = H // G  # heads per core
DL = HL * HD  # 512 local feature dim
P = 128
NT = T // P  # 16 tk tiles
NQ = T // 512  # 4 tq tiles
EC = E // P  # 8 contraction chunks over E
DC = DL // P  # 4 chunks over local head dims

# chunk order matching rope pair arrival (u, u+4)
JSEQ = [0, 4, 1, 5, 2, 6, 3, 7]

_CACHE = {}
LAST_RESULT = None


def _build():
    nc = bacc.Bacc("TRN2", target_bir_lowering=False, debug=False, num_devices=8)
    xt_d = nc.dram_tensor("xt", (EC, P, T), BF, kind="ExternalInput").ap()
    sinh_d = nc.dram_tensor("sinh", (4, P, T), BF, kind="ExternalInput").ap()
    cosh_d = nc.dram_tensor("cosh", (4, P, T), BF, kind="ExternalInput").ap()
    wq_d = nc.dram_tensor("wq", (EC, P, DL), BF, kind="ExternalInput").ap()
    wk_d = nc.dram_tensor("wk", (EC, P, DL), BF, kind="ExternalInput").ap()
    wv_d = nc.dram_tensor("wv", (EC, P, DL), BF, kind="ExternalInput").ap()
    wo_d = nc.dram_tensor("wo", (DC, P, E), BF, kind="ExternalInput").ap()
    wqv = wq_d.rearrange("o p n -> p o n")
    wkv = wk_d.rearrange("o p n -> p o n")
    masks_d = nc.dram_tensor("masks", (P, 512), BF, kind="ExternalInput").ap()
    out_d = nc.dram_tensor("out", (T, E), BF, kind="ExternalOutput").ap()


    with tile.TileContext(nc) as tc:
        with tc.tile_pool(name="persist", bufs=1) as persist:
            rx = persist.tile([P, EC, T], BF)
            qT = persist.tile([P, DC, T], BF)
            kT = persist.tile([P, DC, T], BF)
            v = persist.tile([P, NT, HL, HD + 1], BF)
            oc = persist.tile([P, DC, T], BF)
            wq = persist.tile([P, EC, DL], BF)
            wk = persist.tile([P, EC, DL], BF)
            wv = persist.tile([P, EC, DL], BF)
            masks = persist.tile([P, 512], BF)
            nc.vector.memset(v[:, :, :, HD : HD + 1], 1.0)

            # ---- Phase A: RoPE + V projection, streamed behind the DMAs ----
            with (
                tc.tile_pool(name="xtp", bufs=1) as xtp,
                tc.tile_pool(name="tabs", bufs=1) as tabs,
                tc.tile_pool(name="tmps", bufs=1) as tmps,
                tc.tile_pool(name="mmf", bufs=1, space="PSUM") as mmf,
                tc.tile_pool(name="mm1", bufs=1, space="PSUM") as mm1,
            ):
                filler = mmf.tile([P, 1024], F32)
                xt = xtp.tile([P, EC, T], BF)
                sin_t = tabs.tile([P, 4, T], BF)
                cos_t = tabs.tile([P, 4, T], BF)

                # DMA priority order: wv chunks, (xt pair + tables) packs,
                # masks, per-pair wk/wq, wo (emitted in phase B/C block).
                for u in range(4):
                    nc.sync.dma_start(wv[:, u, :], wv_d[u])
                    nc.sync.dma_start(xt[:, u, :], xt_d[u])
                    nc.sync.dma_start(wv[:, u + 4, :], wv_d[u + 4])
                    nc.sync.dma_start(xt[:, u + 4, :], xt_d[u + 4])
                    nc.sync.dma_start(sin_t[:, u, :], sinh_d[u])
                    nc.sync.dma_start(cos_t[:, u, :], cosh_d[u])
                nc.sync.dma_start(masks, masks_d)
                for hp in range(HL // 2):
                    nc.sync.dma_start(
                        wk[:, :, P * hp : P * (hp + 1)],
                        wkv[:, :, P * hp : P * (hp + 1)],
                    )
                    nc.sync.dma_start(
                        wq[:, :, P * hp : P * (hp + 1)],
                        wqv[:, :, P * hp : P * (hp + 1)],
                    )

                # rope on DVE, one (u, u+4) chunk pair at a time
                for u in range(4):
                    xe = xt[:, u, :]
                    xo = xt[:, u + 4, :]
                    st = sin_t[:, u, :]
                    ct = cos_t[:, u, :]
                    t1 = tmps.tile([P, T], BF, tag="t1")
                    nc.vector.tensor_mul(t1, xe, ct)
                    t2 = tmps.tile([P, T], BF, tag="t2")
                    nc.vector.tensor_mul(t2, xo, st)
                    nc.vector.tensor_sub(rx[:, u, :], t1, t2)
                    t3 = tmps.tile([P, T], BF, tag="t3")
                    nc.vector.tensor_mul(t3, xo, ct)
                    t4 = tmps.tile([P, T], BF, tag="t4")
                    nc.vector.tensor_mul(t4, xe, st)
                    nc.vector.tensor_add(rx[:, u + 4, :], t3, t4)

                # V projection: contraction-outer over 2 groups of 8 tk
                # tiles so each xt chunk is consumed as it arrives
                for grp, tks in enumerate((range(0, 6), range(6, 12), range(12, 16))):
                    vps = {}
                    for tk in tks:
                        vps[tk] = mm1.tile(
                            [P, DL], F32, tag=f"vp{tk % 6}", name=f"vp{tk % 6}"
                        )
                    for idx, j in enumerate(JSEQ):
                        for tk in tks:
                            nc.tensor.matmul(
                                vps[tk],
                                lhsT=xt[:, j, P * tk : P * (tk + 1)],
                                rhs=wv[:, j, :],
                                start=(idx == 0),
                                stop=(idx == 7),
                            )
                    for tk in tks:
                        if tk % 2 == 0:
                            nc.scalar.copy(
                                v[:, tk, :, 0:HD],
                                vps[tk].rearrange("p (h d) -> p h d", h=HL),
                            )
                        else:
                            nc.vector.tensor_copy(
                                v[:, tk, :, 0:HD],
                                vps[tk].rearrange("p (h d) -> p h d", h=HL),
                            )

            # ---- Phase B+C: Q/K projection, attention, output projection ----
            with (
                tc.tile_pool(name="att", bufs=10) as apool,
                tc.tile_pool(name="norm", bufs=2) as npool,
                tc.tile_pool(name="rbp", bufs=1) as rbpool,
                tc.tile_pool(name="wop", bufs=1) as wop,
                tc.tile_pool(name="oout", bufs=3) as oop,
                tc.tile_pool(name="qkps", bufs=2, space="PSUM") as qkps,
                tc.tile_pool(name="ops", bufs=2, space="PSUM") as opool,
                tc.tile_pool(name="sps", bufs=2, space="PSUM") as spool,
                tc.tile_pool(name="dramn", bufs=1, space="DRAM") as dpool,
            ):
                rpk_d = dpool.tile([HL // 2, 8, 512], BF)
                wo = wop.tile([P, DC, E], BF)
                nc.sync.dma_start(wo, wo_d.rearrange("o p n -> p o n"))

                deferred_norm = []
                for hp in range(HL // 2):
                    h0, h1 = 2 * hp, 2 * hp + 1
                    rbtag = f"rbd{hp % 2}" if hp < 3 else "rb3"
                    rb = rbpool.tile([P, T], BF, tag=rbtag, name=f"rb{hp}")
                    if hp < 3:
                        packed_a = npool.tile(
                            [8, 512], BF, tag=f"packed_a{hp}", name=f"packeda{hp}"
                        )
                    else:
                        packed_a = npool.tile(
                            [4, 512], BF, tag="packed_g0", name=f"packedg0{hp}"
                        )
                        packed_g1 = npool.tile(
                            [2, 512], BF, tag="packed_g1", name=f"packedg1{hp}"
                        )
                    packed_b = npool.tile(
                        [2, 512], BF, tag="packed_b", name=f"packedb{hp}"
                    )

                    # Q/K projection for this head pair, ti-outer so the
                    # first attention cell unblocks as early as possible
                    for ti in range(NQ):
                        for w_sb, dst in ((wk, kT), (wq, qT)):
                            pp = qkps.tile([P, 512], F32, tag="qk")
                            for idx, j in enumerate(JSEQ):
                                nc.tensor.matmul(
                                    pp,
                                    lhsT=w_sb[:, j, P * hp : P * (hp + 1)],
                                    rhs=rx[:, j, 512 * ti : 512 * (ti + 1)],
                                    start=(idx == 0),
                                    stop=(idx == 7),
                                )
                            if hp == 0:
                                nc.scalar.copy(
                                    dst[:, hp, 512 * ti : 512 * (ti + 1)], pp
                                )
                            else:
                                nc.vector.tensor_copy(
                                    dst[:, hp, 512 * ti : 512 * (ti + 1)], pp
                                )

                    for i in range(NQ):
                        nj = 4 * i + 4
                        tq = slice(512 * i, 512 * (i + 1))
                        op0 = opool.tile([HD + 1, 512], F32, tag="o")
                        op1 = opool.tile([HD + 1, 512], F32, tag="o")
                        ats = {}

                        def emit_pv(jp, first, last):
                            at_jp = ats[jp]
                            r = jp - 4 * i
                            lo = P * r if r >= 1 else 0
                            nc.tensor.matmul(
                                op0[:, lo:],
                                lhsT=v[:, jp, h0, :],
                                rhs=at_jp[:, 0, lo:],
                                start=first,
                                stop=last,
                            )
                            nc.tensor.matmul(
                                op1[:, lo:],
                                lhsT=v[:, jp, h1, :],
                                rhs=at_jp[:, 1, lo:],
                                start=first,
                                stop=last,
                            )

                        # diagonal blocks first: their suffix exps are small,
                        # so the ACT pipeline ramps quickly at cell start.
                        # PV accumulation runs off-diagonal first (full-width
                        # start) and diagonals last.
                        sorder = list(range(4 * i, nj)) + list(range(0, 4 * i))
                        pvorder = list(range(0, 4 * i)) + list(range(4 * i, nj))
                        npv = 0
                        for idx, j in enumerate(sorder):
                            r = j - 4 * i
                            lo = P * r if r >= 1 else 0
                            sp = spool.tile([P, 2, 512], F32, tag="s")
                            nc.tensor.matmul(
                                sp[:, 0, lo:],
                                lhsT=kT[0:HD, hp, P * j : P * (j + 1)],
                                rhs=qT[0:HD, hp, 512 * i + lo : 512 * (i + 1)],
                                start=True,
                                stop=True,
                            )
                            nc.tensor.matmul(
                                sp[:, 1, lo:],
                                lhsT=kT[HD:P, hp, P * j : P * (j + 1)],
                                rhs=qT[HD:P, hp, 512 * i + lo : 512 * (i + 1)],
                                start=True,
                                stop=True,
                            )
                            at = apool.tile([P, 2, 512], BF, tag="a")
                            if r < 0:
                                nc.scalar.activation(at, sp, EXP, scale=0.125)
                            else:
                                w = 512 - lo
                                nc.scalar.activation(
                                    at[:, :, lo:], sp[:, :, lo:], EXP, scale=0.125
                                )
                                nc.vector.tensor_mul(
                                    at[:, :, lo:],
                                    at[:, :, lo:],
                                    masks[:, None, 0:w].to_broadcast((P, 2, w)),
                                )
                            ats[j] = at
                            # trail PV ~3 ready blocks behind; only emit PVs
                            # whose at exists (off-diags become available
                            # from idx 4 onward)
                            while npv < len(pvorder) and pvorder[npv] in ats:
                                jp = pvorder[npv]
                                if i > 0 and jp >= 4 * i and idx < nj - 1:
                                    break  # diags last; wait until loop end
                                if idx - 3 < npv:
                                    break
                                emit_pv(jp, npv == 0, npv == nj - 1)
                                npv += 1
                        while npv < len(pvorder):
                            emit_pv(pvorder[npv], npv == 0, npv == nj - 1)
                            npv += 1

                        # normalizer rows: copy out, pack via DMA; batch
                        # the expensive DVE reciprocal (cells 0-2 at i==2,
                        # cell 3 alone) and scale oc in two chunks so the
                        # output projection overlaps the pair tail
                        s0 = npool.tile([1, 512], BF, tag="s0")
                        nc.vector.tensor_copy(s0, op0[HD : HD + 1, :])
                        s1 = npool.tile([1, 512], BF, tag="s1")
                        nc.vector.tensor_copy(s1, op1[HD : HD + 1, :])
                        if hp < 3:
                            nc.sync.dma_start(packed_a[i : i + 1, :], s0)
                            nc.sync.dma_start(packed_a[4 + i : 5 + i, :], s1)
                        elif i < 2:
                            nc.sync.dma_start(packed_a[2 * i : 2 * i + 1, :], s0)
                            nc.sync.dma_start(
                                packed_a[2 * i + 1 : 2 * i + 2, :], s1
                            )
                        elif i == 2:
                            nc.sync.dma_start(packed_g1[0:1, :], s0)
                            nc.sync.dma_start(packed_g1[1:2, :], s1)
                        else:
                            nc.sync.dma_start(packed_b[0:1, :], s0)
                            nc.sync.dma_start(packed_b[1:2, :], s1)
                        nc.vector.tensor_copy(oc[0:HD, hp, tq], op0[0:HD, :])
                        nc.vector.tensor_copy(oc[HD:P, hp, tq], op1[0:HD, :])

                        if hp == 3 and i in (1, 2):
                            # last pair: normalize finished cells eagerly so
                            # output-projection tiles unlock as PE filler
                            lo8, hi8 = (0, 4) if i == 1 else (4, 6)
                            gsrc = packed_a if i == 1 else packed_g1
                            rpk_a = npool.tile(
                                [hi8 - lo8, 512],
                                BF,
                                tag=f"rpk_a{i}",
                                name=f"rpka{hp}_{i}",
                            )
                            with nc.allow_low_precision(
                                reason="bf16 softmax normalizer"
                            ):
                                nc.vector.reciprocal(rpk_a, gsrc)
                            nc.sync.dma_start(rpk_d[hp, lo8:hi8], rpk_a)
                            for c in range(lo8 // 2, hi8 // 2):
                                tqi = slice(512 * c, 512 * (c + 1))
                                nc.sync.dma_start(
                                    rb[0:HD, tqi],
                                    rpk_d[hp, 2 * c : 2 * c + 1, :].to_broadcast(
                                        (HD, 512)
                                    ),
                                )
                                nc.sync.dma_start(
                                    rb[HD:P, tqi],
                                    rpk_d[
                                        hp, 2 * c + 1 : 2 * c + 2, :
                                    ].to_broadcast((HD, 512)),
                                )
                            nc.vector.tensor_mul(
                                oc[:, hp, 512 * (lo8 // 2) : 512 * (hi8 // 2)],
                                oc[:, hp, 512 * (lo8 // 2) : 512 * (hi8 // 2)],
                                rb[:, 512 * (lo8 // 2) : 512 * (hi8 // 2)],
                            )

                    if hp < 3:
                        # normalization deferred: emitted just before the
                        # output projection so its DVE cost never outranks
                        # the attention-critical mask multiplies
                        deferred_norm.append((hp, packed_a, rb))
                    else:
                        rpk_b = npool.tile(
                            [2, 512], BF, tag="rpk_b", name=f"rpkb{hp}"
                        )
                        with nc.allow_low_precision(reason="bf16 softmax normalizer"):
                            nc.vector.reciprocal(rpk_b, packed_b)
                        nc.sync.dma_start(rpk_d[hp, 6:8], rpk_b)
                        tq3 = slice(512 * 3, 512 * 4)
                        nc.sync.dma_start(
                            rb[0:HD, tq3],
                            rpk_d[hp, 6:7, :].to_broadcast((HD, 512)),
                        )
                        nc.sync.dma_start(
                            rb[HD:P, tq3],
                            rpk_d[hp, 7:8, :].to_broadcast((HD, 512)),
                        )
                        nc.vector.tensor_mul(
                            oc[:, hp, tq3], oc[:, hp, tq3], rb[:, tq3]
                        )

                for dhp, dpacked, drb in deferred_norm:
                    rpk_a = npool.tile(
                        [8, 512], BF, tag=f"rpk_a8_{dhp}", name=f"rpka8{dhp}"
                    )
                    with nc.allow_low_precision(reason="bf16 softmax normalizer"):
                        nc.vector.reciprocal(rpk_a, dpacked)
                    nc.sync.dma_start(rpk_d[dhp, 0:8], rpk_a)
                    for ii in range(4):
                        tqi = slice(512 * ii, 512 * (ii + 1))
                        nc.sync.dma_start(
                            drb[0:HD, tqi],
                            rpk_d[dhp, ii : ii + 1, :].to_broadcast((HD, 512)),
                        )
                        nc.sync.dma_start(
                            drb[HD:P, tqi],
                            rpk_d[dhp, 4 + ii : 5 + ii, :].to_broadcast(
                                (HD, 512)
                            ),
                        )
                    nc.vector.tensor_mul(oc[:, dhp, :], oc[:, dhp, :], drb)

                # ---- output projection (shares the qk PSUM pool; the
                # dataflow scheduler overlaps it with the last pair) ----
                for tt in range(NT):
                    ot = oop.tile([P, E], BF, tag="ot")
                    for et in range(2):
                        pp = qkps.tile([P, 512], F32, tag="qk")
                        for kk in range(DC):
                            nc.tensor.matmul(
                                pp,
                                lhsT=oc[:, kk, P * tt : P * (tt + 1)],
                                rhs=wo[:, kk, 512 * et : 512 * (et + 1)],
                                start=(kk == 0),
                                stop=(kk == DC - 1),
                            )
                        if et == 0:
                            nc.scalar.copy(ot[:, 512 * et : 512 * (et + 1)], pp)
                        else:
                            nc.vector.tensor_copy(
                                ot[:, 512 * et : 512 * (et + 1)], pp
                            )
                        nc.sync.dma_start(
                            out_d[
                                P * tt : P * (tt + 1), 512 * et : 512 * (et + 1)
                            ],
                            ot[:, 512 * et : 512 * (et + 1)],
                        )

    nc.compile()
    nc.m = get_hw_module(nc.m)
    return nc


def _prep_inputs(input, Wq, Wk, Wv, Wo):
    """Host-side shard prep: transpose/de-interleave/cast. Returns 8 in_maps."""
    perm = np.concatenate([np.arange(0, E, 2), np.arange(1, E, 2)])

    u = np.arange(E // 2, dtype=np.float64)
    thetas = 10000.0 ** (-2.0 * u / E)
    ang = np.arange(T, dtype=np.float64)[:, None] * thetas[None, :]
    sinh = np.sin(ang).T.reshape(4, P, T).astype(bf16)
    cosh = np.cos(ang).T.reshape(4, P, T).astype(bf16)

    masks = np.zeros((P, 512), np.float32)
    f = np.arange(512)
    for p in range(P):
        masks[p] = (f >= p).astype(np.float32)
    masks = masks.astype(bf16)

    xt = [
        np.ascontiguousarray(input[b].T[perm]).reshape(EC, P, T).astype(bf16)
        for b in range(B)
    ]
    WqT, WkT, WvT = Wq.T[perm], Wk.T[perm], Wv.T[perm]
    wq_g = [
        np.ascontiguousarray(WqT[:, DL * g : DL * (g + 1)])
        .reshape(EC, P, DL)
        .astype(bf16)
        for g in range(G)
    ]
    wk_g = [
        np.ascontiguousarray(WkT[:, DL * g : DL * (g + 1)])
        .reshape(EC, P, DL)
        .astype(bf16)
        for g in range(G)
    ]
    wv_g = [
        np.ascontiguousarray(WvT[:, DL * g : DL * (g + 1)])
        .reshape(EC, P, DL)
        .astype(bf16)
        for g in range(G)
    ]
    wo_g = [
        np.ascontiguousarray(Wo.T[DL * g : DL * (g + 1)])
        .reshape(DC, P, E)
        .astype(bf16)
        for g in range(G)
    ]

    in_maps = []
    for c in range(8):
        b, g = c // 2, c % 2
        in_maps.append(
            {
                "xt": xt[b],
                "sinh": sinh,
                "cosh": cosh,
                "wq": wq_g[g],
                "wk": wk_g[g],
                "wv": wv_g[g],
                "wo": wo_g[g],
                "masks": masks,
            }
        )
    return in_maps


def kernel(input, Wq, Wk, Wv, Wo, bo):
    global LAST_RESULT
    input = np.asarray(input, np.float32)
    Wq, Wk, Wv, Wo = (np.asarray(w, np.float32) for w in (Wq, Wk, Wv, Wo))
    bo = np.asarray(bo, np.float32)

    if "nc" not in _CACHE:
        _CACHE["nc"] = _build()
    nc = _CACHE["nc"]

    in_maps = _prep_inputs(input, Wq, Wk, Wv, Wo)
    res = bass_utils.run_bass_kernel_spmd(nc, in_maps, core_ids=list(range(8)))
    LAST_RESULT = res

    out = np.empty((B, T, E), np.float32)
    for b in range(B):
        out[b] = (
            res.results[2 * b]["out"].astype(np.float32)
            + res.results[2 * b + 1]["out"].astype(np.float32)
            + bo
        )
    return out
